# revision 1
# baseline (speedup 1.0000x reference)
"""Deformable conv block (3x3, offsets from a conv) on 8 TRN2 NeuronCores.

Self-contained: kernel(**inputs) takes full numpy inputs, shards
data-parallel over (batch, H-half) across 8 cores, runs one SPMD Bass
program per core via run_bass_kernel_spmd, and reassembles the full
output. All FLOPs (offset conv, bilinear sampling via GPSIMD
indirect_copy gather, main conv) run on device.
"""
import numpy as np

import concourse.bass as bass
import concourse.mybir as mybir
import concourse.tile as tile_mod
from concourse import tile
from concourse.vector_clock import ScopedClock

# ---------------------------------------------------------------------------
# Patch 1: this container's walrus accepts at most ONE sync wait per
# instruction; split the tile-exit drain's waits across preceding SP nops.
def _drain_and_barrier(self, tick_clock, wait_clock):
    nc = self.nc
    carriers = [nc.sync.nop(nofuse=True, hint=f"drainwait{i}") for i in range(32)]
    drain_inst = nc.sync.drain()
    wait_clock.add_sem_waits(drain_inst.ins, ScopedClock({None: tick_clock.global_clock}))
    si = drain_inst.ins.sync_info
    waits = list(si.on_wait or [])
    if len(waits) > 1:
        si.on_wait = waits[:1]
        for i, w in enumerate(waits[1:]):
            ci = carriers[i].ins
            if ci.sync_info is None:
                ci.sync_info = mybir.SyncInfo(on_wait=[w], on_update=[])
            else:
                ci.sync_info.on_wait = (ci.sync_info.on_wait or []) + [w]
    nc.all_engine_barrier()
    assert self.sems is not None
    popped = nc._tile_sem_poison_stack.pop()
    assert popped is self._sem_poison
    nc.clear_and_free_semaphores(list(self.sems.allocated().values()))
    nc.all_engine_barrier()

tile_mod.TileContext._drain_and_barrier = _drain_and_barrier

# Patch 2: split multi-wait instructions everywhere (same walrus limit).
_ctr = [0]

def _mk_nop(engine, wait):
    _ctr[0] += 1
    nop = mybir.InstNoOp(name=f"WSPLIT-{_ctr[0]}", ins=[], outs=[])
    nop.engine = engine
    nop.sync_info = mybir.SyncInfo(on_wait=[wait], on_update=[])
    return nop

def split_waits(nc):
    n = 0
    for fn in nc.m.functions:
        for bb in fn.blocks:
            insts = list(bb.instructions)
            outl, changed = [], False
            for inst in insts:
                si = inst.sync_info
                if si is not None and si.on_wait and len(si.on_wait) > 1:
                    waits = list(si.on_wait)
                    for w in waits[:-1]:
                        nop = _mk_nop(inst.engine, w)
                        nc.register_instruction(nop, overwrite=True)
                        outl.append(nop)
                        n += 1
                    si.on_wait = waits[-1:]
                    inst.sync_info = si
                    changed = True
                outl.append(inst)
            if changed:
                bb.instructions = outl
    return n

# ---------------------------------------------------------------------------
F32 = mybir.dt.float32
I32 = mybir.dt.int32
U16 = mybir.dt.uint16
AO = mybir.AluOpType
AP = bass.AP

B, Cin, Cout, H, W = 4, 64, 64, 128, 128
KK = 9
PADW = 133              # padded cols: x+2 for x in [-2, 130]
SLABROWS = 84           # slab rows: global-padded h0-8 .. h0+75
RBLK = 8
NBLK = 8
WIN_ROWS = 28
WIN = WIN_ROWS * PADW   # 3724
NS = RBLK * W           # 1024 samples per (block, tap)
NPIX = 64 * W
IWF = KK * NBLK * 64    # wrapped-idx free size per partition (4608)


def build_program():
    nc = bass.Bass()
    xslab = nc.declare_dram_parameter("xslab", [Cin, SLABROWS * PADW], F32, isOutput=False)
    cl_lo = nc.declare_dram_parameter("cl_lo", [128, 64], F32, isOutput=False)
    cl_hi = nc.declare_dram_parameter("cl_hi", [128, 64], F32, isOutput=False)
    baseY = nc.declare_dram_parameter("baseY", [128, 64], F32, isOutput=False)
    baseX = nc.declare_dram_parameter("baseX", [128, 64], F32, isOutput=False)
    ident = nc.declare_dram_parameter("ident", [128, 128], F32, isOutput=False)
    offwT = nc.declare_dram_parameter("offwT", [Cin, KK * 18], F32, isOutput=False)
    mainWT = nc.declare_dram_parameter("mainWT", [128, KK * Cout], F32, isOutput=False)
    offb = nc.declare_dram_parameter("offb", [18, 1], F32, isOutput=False)
    mainb = nc.declare_dram_parameter("mainb", [Cout, 1], F32, isOutput=False)
    ohA = nc.declare_dram_parameter("ohA", [4, 128], F32, isOutput=False)
    ohB = nc.declare_dram_parameter("ohB", [4, 128], F32, isOutput=False)
    out = nc.declare_dram_parameter("out", [Cout, NPIX], F32, isOutput=True)
    dbg_offs = nc.declare_dram_parameter("dbg_offs", [18, NPIX], F32, isOutput=True)
    dbg_idx = nc.declare_dram_parameter("dbg_idx", [128, KK * 64 * 2], U16, isOutput=True)

    with tile.TileContext(nc) as tc:
        with (
            tc.tile_pool(name="big", bufs=1) as big,
            tc.tile_pool(name="ps", bufs=4, space="PSUM") as ps,
            tc.tile_pool(name="psacc", bufs=2, space="PSUM") as psa,
            tc.tile_pool(name="work", bufs=2) as wk,
            tc.tile_pool(name="g", bufs=4) as gp,
        ):
            xs = big.tile([128, SLABROWS * PADW], F32, tag="xs")
            nc.sync.dma_start(xs[0:64, :], xslab[:, :])
            nc.sync.dma_start(xs[64:128, :], xslab[:, :])
            woff = big.tile([Cin, KK * 18], F32, tag="woff")
            nc.sync.dma_start(woff[:, :], offwT[:, :])
            wmain = big.tile([128, KK * Cout], F32, tag="wmain")
            nc.sync.dma_start(wmain[:, :], mainWT[:, :])
            bY = big.tile([128, 64], F32, tag="bY")
            nc.sync.dma_start(bY[:, :], baseY[:, :])
            bX = big.tile([128, 64], F32, tag="bX")
            nc.sync.dma_start(bX[:, :], baseX[:, :])
            cLo = big.tile([128, 64], F32, tag="cLo")
            nc.sync.dma_start(cLo[:, :], cl_lo[:, :])
            cHi = big.tile([128, 64], F32, tag="cHi")
            nc.sync.dma_start(cHi[:, :], cl_hi[:, :])
            idn = big.tile([128, 128], F32, tag="idn")
            nc.sync.dma_start(idn[:, :], ident[:, :])
            ob = big.tile([18, 1], F32, tag="ob")
            nc.sync.dma_start(ob[:, :], offb[:, :])
            mb = big.tile([Cout, 1], F32, tag="mb")
            nc.sync.dma_start(mb[:, :], mainb[:, :])
            ohAt = big.tile([4, 128], F32, tag="ohAt")
            nc.sync.dma_start(ohAt[:, :], ohA[:, :])
            ohBt = big.tile([4, 128], F32, tag="ohBt")
            nc.sync.dma_start(ohBt[:, :], ohB[:, :])

            # ---- 1. offset conv ----
            offs = big.tile([18, NPIX], F32, tag="offs")
            for ch in range(16):
                pt = ps.tile([18, 512], F32, tag="pp")
                h0c = ch * 4
                for t in range(KK):
                    ki, kj = t // 3, t % 3
                    off0 = (h0c + ki - 1 + 10) * PADW + (kj - 1 + 2)
                    rhs = AP(xs[:].tensor, xs[:].offset + off0,
                             [[SLABROWS * PADW, 64], [PADW, 4], [1, 128]])
                    nc.tensor.matmul(pt[:, :], woff[:, t * 18:(t + 1) * 18], rhs,
                                     start=(t == 0), stop=(t == KK - 1))
                nc.vector.tensor_scalar(offs[:, ch * 512:(ch + 1) * 512], pt[:, :],
                                        ob[:, 0:1], None, AO.add)
            nc.sync.dma_start(dbg_offs[:, :], offs[:, :])

            # ---- 2. transpose offsets -> offT [128w, (64h, 18)] ----
            offT = big.tile([128, 64 * 18], F32, tag="offT")
            for h in range(64):
                tp = ps.tile([128, 18], F32, tag="pp")
                nc.tensor.transpose(tp[:, :], offs[:, h * 128:(h + 1) * 128],
                                    idn[0:18, 0:18])
                ov = AP(offT[:].tensor, offT[:].offset + h * 18,
                        [[64 * 18, 128], [1, 18]])
                nc.vector.tensor_copy(ov, tp[:, :])

            # ---- 3. per-tap pipeline -> wcc, idxP ----
            wcc = big.tile([128, KK * 4 * 64], F32, tag="wcc")
            idxP = big.tile([128, KK * 64 * 2], U16, tag="idxP")
            for t in range(KK):
                ki, kj = t // 3, t % 3
                dy = AP(offT[:].tensor, offT[:].offset + 2 * t,
                        [[64 * 18, 128], [18, 64]])
                dx = AP(offT[:].tensor, offT[:].offset + 2 * t + 1,
                        [[64 * 18, 128], [18, 64]])
                py = wk.tile([128, 64], F32, tag="py")
                px = wk.tile([128, 64], F32, tag="px")
                nc.vector.tensor_tensor(py[:, :], dy, bY[:, :], AO.add)
                nc.vector.tensor_scalar(py[:, :], py[:, :], float(ki - 1), None, AO.add)
                nc.vector.tensor_tensor(py[:, :], py[:, :], cLo[:, :], AO.max)
                nc.vector.tensor_tensor(py[:, :], py[:, :], cHi[:, :], AO.min)
                nc.vector.tensor_tensor(px[:, :], dx, bX[:, :], AO.add)
                nc.vector.tensor_scalar(px[:, :], px[:, :], float(kj - 1), None, AO.add)
                nc.vector.tensor_scalar(px[:, :], px[:, :], -2.0, 129.0, AO.max, AO.min)
                y0i = wk.tile([128, 64], I32, tag="y0i")
                x0i = wk.tile([128, 64], I32, tag="x0i")
                y0f = wk.tile([128, 64], F32, tag="y0f")
                x0f = wk.tile([128, 64], F32, tag="x0f")
                tmp = wk.tile([128, 64], F32, tag="tmp")
                nc.vector.tensor_scalar(tmp[:, :], py[:, :], 0.5, None, AO.subtract)
                nc.vector.tensor_copy(y0i[:, :], tmp[:, :])
                nc.vector.tensor_copy(y0f[:, :], y0i[:, :])
                nc.vector.tensor_scalar(tmp[:, :], px[:, :], 0.5, None, AO.subtract)
                nc.vector.tensor_copy(x0i[:, :], tmp[:, :])
                nc.vector.tensor_copy(x0f[:, :], x0i[:, :])
                ly = wk.tile([128, 64], F32, tag="ly")
                lx = wk.tile([128, 64], F32, tag="lx")
                my = wk.tile([128, 64], F32, tag="my")
                mx = wk.tile([128, 64], F32, tag="mx")
                nc.vector.tensor_tensor(ly[:, :], py[:, :], y0f[:, :], AO.subtract)
                nc.vector.tensor_tensor(lx[:, :], px[:, :], x0f[:, :], AO.subtract)
                nc.vector.tensor_scalar(my[:, :], ly[:, :], -1.0, 1.0, AO.mult, AO.add)
                nc.vector.tensor_scalar(mx[:, :], lx[:, :], -1.0, 1.0, AO.mult, AO.add)
                for r, (a, bb) in enumerate([(my, mx), (my, lx), (ly, mx), (ly, lx)]):
                    wv = AP(wcc[:].tensor, wcc[:].offset + (t * 4 + r) * 64,
                            [[KK * 4 * 64, 128], [1, 64]])
                    nc.vector.tensor_tensor(wv, a[:, :], bb[:, :], AO.mult)
                nc.vector.tensor_scalar(x0f[:, :], x0f[:, :], 2.0, None, AO.add)
                for blk in range(NBLK):
                    hb = blk * RBLK
                    sl = slice(hb, hb + RBLK)
                    tb = wk.tile([128, RBLK], F32, tag="tb")
                    nc.vector.tensor_scalar(tb[:, :], y0f[:, sl], float(10 - hb),
                                            133.0, AO.add, AO.mult)
                    nc.vector.tensor_tensor(tb[:, :], tb[:, :], x0f[:, sl], AO.add)
                    nc.vector.tensor_scalar(tb[:, :], tb[:, :], 0.0,
                                            float(WIN - 135), AO.max, AO.min)
                    iA = AP(idxP[:].tensor, idxP[:].offset + (t * 64 + hb) * 2,
                            [[KK * 64 * 2, 128], [2, RBLK]])
                    nc.vector.tensor_copy(iA, tb[:, :])
                    nc.vector.tensor_scalar(tb[:, :], tb[:, :], 133.0, None, AO.add)
                    iB = AP(idxP[:].tensor, idxP[:].offset + (t * 64 + hb) * 2 + 1,
                            [[KK * 64 * 2, 128], [2, RBLK]])
                    nc.vector.tensor_copy(iB, tb[:, :])
            nc.sync.dma_start(dbg_idx[:, :], idxP[:, :])

            # ---- 4. rewrap idx: iw[16g+k, (t, blk, hh*8+m8)] ----
            iw = big.tile([128, IWF], U16, tag="iw")
            for m8 in range(8):
                for g4 in range(4):
                    for ab in range(2):
                        dst = AP(iw[:].tensor,
                                 iw[:].offset + (64 * ab + 16 * g4) * IWF + m8,
                                 [[IWF, 16], [NBLK * 64, KK], [64, NBLK], [8, RBLK]])
                        src = AP(idxP[:].tensor,
                                 idxP[:].offset + (16 * m8) * (KK * 64 * 2) + ab,
                                 [[KK * 64 * 2, 16], [128, KK], [16, NBLK], [2, RBLK]])
                        nc.sync.dma_start(dst, src)

            # ---- 5/6/7 per block ----
            for blk in range(NBLK):
                hb = blk * RBLK
                pt3a = psa.tile([Cout, 512], F32, tag="acc")
                pt3b = psa.tile([Cout, 512], F32, tag="acc")
                for t in range(KK):
                    gA = gp.tile([128, NS], F32, tag="gA")
                    gB = gp.tile([128, NS], F32, tag="gB")
                    iview = AP(iw[:].tensor, iw[:].offset + (t * NBLK + blk) * 64,
                               [[IWF, 128], [1, 64]])
                    win0 = hb * PADW
                    dataA = AP(xs[:].tensor, xs[:].offset + win0,
                               [[SLABROWS * PADW, 128], [1, WIN - 1], [1, 1]])
                    dataB = AP(xs[:].tensor, xs[:].offset + win0 + 1,
                               [[SLABROWS * PADW, 128], [1, WIN - 1], [1, 1]])
                    nc.gpsimd.indirect_copy(
                        gA[:].rearrange("p (n i) -> p n i", i=1), dataA, iview, True)
                    nc.gpsimd.indirect_copy(
                        gB[:].rearrange("p (n i) -> p n i", i=1), dataB, iview, True)
                    wcmp = wk.tile([4, NS], F32, tag="wcmp")
                    for hh in range(RBLK):
                        tp2 = ps.tile([4, 128], F32, tag="pp")
                        wsl = AP(wcc[:].tensor,
                                 wcc[:].offset + (t * 4) * 64 + (hb + hh),
                                 [[KK * 4 * 64, 128], [64, 4]])
                        nc.tensor.transpose(tp2[:, :], wsl, idn[:, :])
                        nc.vector.tensor_copy(wcmp[:, hh * 128:(hh + 1) * 128],
                                              tp2[:, :])
                    for half in range(2):
                        cs = slice(half * 512, (half + 1) * 512)
                        wra = ps.tile([128, 512], F32, tag="pp")
                        nc.tensor.matmul(wra[:, :], ohAt[:, :], wcmp[:, cs],
                                         start=True, stop=True)
                        nc.vector.tensor_tensor(gA[:, cs], gA[:, cs], wra[:, :],
                                                AO.mult)
                        wrb = ps.tile([128, 512], F32, tag="pp")
                        nc.tensor.matmul(wrb[:, :], ohBt[:, :], wcmp[:, cs],
                                         start=True, stop=True)
                        nc.vector.tensor_tensor(gB[:, cs], gB[:, cs], wrb[:, :],
                                                AO.mult)
                    # accumulate into main-conv PSUM (K=128 dup'd weights)
                    wsl2 = wmain[:, t * Cout:(t + 1) * Cout]
                    nc.tensor.matmul(pt3a[:, :], wsl2, gA[:, 0:512],
                                     start=(t == 0), stop=False)
                    nc.tensor.matmul(pt3a[:, :], wsl2, gB[:, 0:512],
                                     start=False, stop=(t == KK - 1))
                    nc.tensor.matmul(pt3b[:, :], wsl2, gA[:, 512:1024],
                                     start=(t == 0), stop=False)
                    nc.tensor.matmul(pt3b[:, :], wsl2, gB[:, 512:1024],
                                     start=False, stop=(t == KK - 1))
                for nchunk, pt3 in ((0, pt3a), (1, pt3b)):
                    ot = wk.tile([Cout, 512], F32, tag="ot")
                    nc.vector.tensor_scalar(ot[:, :], pt3[:, :], mb[:, 0:1], None,
                                            AO.add)
                    nc.sync.dma_start(
                        out[:, blk * NS + nchunk * 512:blk * NS + (nchunk + 1) * 512],
                        ot[:, :])
    return nc


def make_host_consts():
    """Input-independent constants shared by all cores."""
    c = {}
    c["baseY"] = np.tile(np.arange(64, dtype=np.float32)[None, :], (128, 1))
    c["baseX"] = np.tile(np.arange(128, dtype=np.float32)[:, None], (1, 64))
    c["ident"] = np.eye(128, dtype=np.float32)
    ohA = np.zeros((4, 128), np.float32)
    ohA[0, 0:64] = 1.0
    ohA[2, 64:128] = 1.0
    ohB = np.zeros((4, 128), np.float32)
    ohB[1, 0:64] = 1.0
    ohB[3, 64:128] = 1.0
    c["ohA"], c["ohB"] = ohA, ohB
    return c


def make_in_maps(x, offset_w, offset_b, weight, bias):
    consts = make_host_consts()
    offwT = np.ascontiguousarray(
        offset_w.reshape(18, Cin, KK).transpose(1, 2, 0)).reshape(Cin, KK * 18)
    mwt = np.ascontiguousarray(
        weight.reshape(Cout, Cin, KK).transpose(1, 2, 0)).reshape(Cin, KK * Cout)
    mainWT = np.concatenate([mwt, mwt], axis=0)
    consts["offwT"] = offwT.astype(np.float32)
    consts["mainWT"] = mainWT.astype(np.float32)
    consts["offb"] = offset_b.reshape(18, 1).astype(np.float32)
    consts["mainb"] = bias.reshape(Cout, 1).astype(np.float32)
    # padded image per batch: [Cin, 133, 133], zeros border (+2 top/left, +3 bot/right)
    xpad = np.zeros((B, Cin, PADW, PADW), np.float32)
    xpad[:, :, 2:2 + H, 2:2 + W] = x
    in_maps = []
    for core in range(8):
        b, half = core // 2, core % 2
        h0 = half * 64
        # slab rows: global-padded rows h0-8 .. h0+75 (84 rows), zero-filled OOB
        slab = np.zeros((Cin, SLABROWS, PADW), np.float32)
        glo = h0 - 8
        lo = max(0, glo)
        hi = min(PADW, glo + SLABROWS)
        slab[:, lo - glo:hi - glo, :] = xpad[b, :, lo:hi, :]
        m = dict(consts)
        m["xslab"] = slab.reshape(Cin, SLABROWS * PADW)
        m["cl_lo"] = np.full((128, 64), -2.0 - h0, np.float32)
        m["cl_hi"] = np.full((128, 64), 129.0 - h0, np.float32)
        in_maps.append(m)
    return in_maps


_CACHED = {}

def kernel(x, offset_w, offset_b, weight, bias):
    from concourse.bass_utils import run_bass_kernel_spmd
    x = np.asarray(x, dtype=np.float32)
    offset_w = np.asarray(offset_w, dtype=np.float32)
    offset_b = np.asarray(offset_b, dtype=np.float32)
    weight = np.asarray(weight, dtype=np.float32)
    bias = np.asarray(bias, dtype=np.float32)
    if "nc" not in _CACHED:
        nc = build_program()
        split_waits(nc)
        _CACHED["nc"] = nc
    nc = _CACHED["nc"]
    in_maps = make_in_maps(x, offset_w, offset_b, weight, bias)
    res = run_bass_kernel_spmd(nc, in_maps, list(range(8)))
    out = np.zeros((B, Cout, H, W), dtype=np.float32)
    for core in range(8):
        b, half = core // 2, core % 2
        out[b, :, half * 64:(half + 1) * 64, :] = (
            res.results[core]["out"].reshape(Cout, 64, W))
    return out



# revision 2
# speedup vs baseline: 1.9087x; 1.9087x over previous
"""Deformable conv block (3x3) on 8 TRN2 NeuronCores — v2.1c.

Per core: one (batch, H-half): 64 output rows x 128 cols.
Key changes vs baseline:
  - all matmuls bf16 (4x PE speedup vs fp32)
  - row-pair-expanded slab: P2[c, 2j] = s[j], P2[c, 2j+1] = s[j+133], so one
    4-byte DGE descriptor fetches a (top, bottom) corner pair
  - one indirect_copy per (tap, quarter): partitions 0:63 fetch the left
    corner pair at idx, 64:127 the right pair at idx+2 -> K=128 main matmuls
  - wrapped gather indices built on PE (transpose + one-hot fold matmuls),
    no tiny strided SBUF->SBUF DMAs
  - all-taps vectorized coordinate math on DVE ([128,576] ops)
  - bilinear weights routed to pixel-major via one DRAM bounce
"""
import numpy as np

import concourse.bass as bass
import concourse.mybir as mybir
import concourse.tile as tile_mod
from concourse import tile
from concourse.vector_clock import ScopedClock

# ---------------------------------------------------------------------------
# Patch 1: this container's walrus accepts at most ONE sync wait per
# instruction; split the tile-exit drain's waits across preceding SP nops.
def _drain_and_barrier(self, tick_clock, wait_clock):
    nc = self.nc
    carriers = [nc.sync.nop(nofuse=True, hint=f"drainwait{i}") for i in range(32)]
    drain_inst = nc.sync.drain()
    wait_clock.add_sem_waits(drain_inst.ins, ScopedClock({None: tick_clock.global_clock}))
    si = drain_inst.ins.sync_info
    waits = list(si.on_wait or [])
    if len(waits) > 1:
        si.on_wait = waits[:1]
        for i, w in enumerate(waits[1:]):
            ci = carriers[i].ins
            if ci.sync_info is None:
                ci.sync_info = mybir.SyncInfo(on_wait=[w], on_update=[])
            else:
                ci.sync_info.on_wait = (ci.sync_info.on_wait or []) + [w]
    nc.all_engine_barrier()
    assert self.sems is not None
    popped = nc._tile_sem_poison_stack.pop()
    assert popped is self._sem_poison
    nc.clear_and_free_semaphores(list(self.sems.allocated().values()))
    nc.all_engine_barrier()

tile_mod.TileContext._drain_and_barrier = _drain_and_barrier

# Patch 2: split multi-wait instructions everywhere (same walrus limit).
_ctr = [0]

def _mk_nop(engine, wait):
    _ctr[0] += 1
    nop = mybir.InstNoOp(name=f"WSPLIT-{_ctr[0]}", ins=[], outs=[])
    nop.engine = engine
    nop.sync_info = mybir.SyncInfo(on_wait=[wait], on_update=[])
    return nop

def split_waits(nc):
    n = 0
    for fn in nc.m.functions:
        for bb in fn.blocks:
            insts = list(bb.instructions)
            outl, changed = [], False
            for inst in insts:
                si = inst.sync_info
                if si is not None and si.on_wait and len(si.on_wait) > 1:
                    waits = list(si.on_wait)
                    for w in waits[:-1]:
                        nop = _mk_nop(inst.engine, w)
                        nc.register_instruction(nop, overwrite=True)
                        outl.append(nop)
                        n += 1
                    si.on_wait = waits[-1:]
                    inst.sync_info = si
                    changed = True
                outl.append(inst)
            if changed:
                bb.instructions = outl
    return n

# ---------------------------------------------------------------------------
F32 = mybir.dt.float32
BF16 = mybir.dt.bfloat16
I32 = mybir.dt.int32
U16 = mybir.dt.uint16
AO = mybir.AluOpType
AP = bass.AP

B, Cin, Cout, H, W = 4, 64, 64, 128, 128
KK = 9
PADW = 133
SLABROWS = 84
SLAB = SLABROWS * PADW          # 11172 slab elements per channel
NPIX = 64 * W                   # 8192 output pixels per core
NT = 576                        # 64 h x 9 taps coordinate lanes
NQ = 4                          # quarters of the pixel space (by w)
QS = NPIX // NQ                 # 2048 slots per quarter
CHUNK = 512                     # slots per PSUM accumulator chunk
NCH = QS // CHUNK               # 4 chunks per quarter
WTAP = 2 * NPIX * 2             # weight-dram elements per tap (2 planes x 2)


def build_program():
    nc = bass.Bass()
    Pp = nc.declare_dram_parameter("P", [Cin, 2 * SLAB], BF16, isOutput=False)
    woff = nc.declare_dram_parameter("woff", [Cin, KK * 18], BF16, isOutput=False)
    wmain = nc.declare_dram_parameter("wmain", [128, KK * Cout], BF16, isOutput=False)
    bYp = nc.declare_dram_parameter("bYp", [128, NT], F32, isOutput=False)
    bXp = nc.declare_dram_parameter("bXp", [128, NT], F32, isOutput=False)
    idnf = nc.declare_dram_parameter("idnf", [128, 128], F32, isOutput=False)
    idnb = nc.declare_dram_parameter("idnb", [128, 128], BF16, isOutput=False)
    FOLD = nc.declare_dram_parameter("FOLD", [64, 4 * 128], F32, isOutput=False)
    OH2 = nc.declare_dram_parameter("OH2", [2, 128], BF16, isOutput=False)
    offb = nc.declare_dram_parameter("offb", [18, 1], F32, isOutput=False)
    mainb = nc.declare_dram_parameter("mainb", [Cout, 1], F32, isOutput=False)
    clo = nc.declare_dram_parameter("clo", [128, 1], F32, isOutput=False)
    chi = nc.declare_dram_parameter("chi", [128, 1], F32, isOutput=False)
    out = nc.declare_dram_parameter("out", [Cout, NPIX], F32, isOutput=True)

    with tile.TileContext(nc) as tc:
        with (
            tc.tile_pool(name="big", bufs=1) as big,
            tc.tile_pool(name="g", bufs=3) as gp,
            tc.tile_pool(name="wl", bufs=2) as wl,
            tc.tile_pool(name="dr", bufs=1, space="DRAM") as dr,
        ):
            Pt = big.tile([128, 2 * SLAB], BF16, tag="Pt")
            nc.sync.dma_start(Pt[0:64, :], Pp[:, :])
            nc.sync.dma_start(Pt[64:128, :], Pp[:, :])
            woffT = big.tile([Cin, KK * 18], BF16, tag="woff")
            nc.sync.dma_start(woffT[:, :], woff[:, :])
            wmainT = big.tile([128, KK * Cout], BF16, tag="wmain")
            nc.sync.dma_start(wmainT[:, :], wmain[:, :])
            bY = big.tile([128, NT], F32, tag="bY")
            nc.sync.dma_start(bY[:, :], bYp[:, :])
            bX = big.tile([128, NT], F32, tag="bX")
            nc.sync.dma_start(bX[:, :], bXp[:, :])
            idnF = big.tile([128, 128], F32, tag="idnF")
            nc.sync.dma_start(idnF[:, :], idnf[:, :])
            idnB = big.tile([128, 128], BF16, tag="idnB")
            nc.sync.dma_start(idnB[:, :], idnb[:, :])
            FOLDt = big.tile([64, 4 * 128], F32, tag="FOLD")
            nc.sync.dma_start(FOLDt[:, :], FOLD[:, :])
            OH2t = big.tile([2, 128], BF16, tag="OH2")
            nc.sync.dma_start(OH2t[:, :], OH2[:, :])
            obt = big.tile([18, 1], F32, tag="ob")
            nc.sync.dma_start(obt[:, :], offb[:, :])
            mbt = big.tile([Cout, 1], F32, tag="mb")
            nc.sync.dma_start(mbt[:, :], mainb[:, :])
            clot = big.tile([128, 1], F32, tag="clo")
            nc.sync.dma_start(clot[:, :], clo[:, :])
            chit = big.tile([128, 1], F32, tag="chi")
            nc.sync.dma_start(chit[:, :], chi[:, :])

            head = tc.tile_pool(name="psh", bufs=2, space="PSUM")
            ps = head.__enter__()
            wkpool = tc.tile_pool(name="wk", bufs=1)
            wk = wkpool.__enter__()

            # ---- 1. offset conv -> offs bf16 [18, 8192] (pixel-major h*128+w)
            # x values sit at even offsets of the row-pair slab (stride 2)
            offs = big.tile([18, NPIX], BF16, tag="offs")
            for ch in range(16):
                pt = ps.tile([18, 512], F32, tag="pofs")
                for t in range(KK):
                    ki, kj = t // 3, t % 3
                    off0 = 2 * ((ch * 4 + ki + 7) * PADW + (kj + 1))
                    rhs = AP(Pt[:].tensor, Pt[:].offset + off0,
                             [[2 * SLAB, 64], [2 * PADW, 4], [2, 128]])
                    nc.tensor.matmul(pt[:, :], woffT[:, t * 18:(t + 1) * 18], rhs,
                                     start=(t == 0), stop=(t == KK - 1))
                nc.scalar.activation(offs[:, ch * 512:(ch + 1) * 512], pt[:, :],
                                     mybir.ActivationFunctionType.Identity,
                                     bias=obt[:, 0:1])

            # ---- 2. transpose offsets -> offT f32 [128w, (64h x 18ch)]
            offT = big.tile([128, 64 * 18], F32, tag="offT")
            for h in range(64):
                tp = ps.tile([128, 18], BF16, tag="ptr")
                nc.tensor.transpose(tp[:, :], offs[:, h * 128:(h + 1) * 128],
                                    idnB[0:18, 0:18])
                ov = AP(offT[:].tensor, offT[:].offset + h * 18,
                        [[64 * 18, 128], [1, 18]])
                nc.scalar.copy(ov, tp[:, :])

            # ---- 3. all-taps coordinate pipeline on [128, 576] (f = h*9 + t)
            dyv = AP(offT[:].tensor, offT[:].offset + 0, [[64 * 18, 128], [18, 64], [2, 9]])
            dxv = AP(offT[:].tensor, offT[:].offset + 1, [[64 * 18, 128], [18, 64], [2, 9]])
            py = wk.tile([128, NT], F32, tag="py")
            px = wk.tile([128, NT], F32, tag="px")
            tmp = wk.tile([128, NT], F32, tag="tmp")
            y0i = wk.tile([128, NT], I32, tag="y0i")
            x0i = wk.tile([128, NT], I32, tag="x0i")
            y0f = wk.tile([128, NT], F32, tag="y0f")
            x0f = wk.tile([128, NT], F32, tag="x0f")
            ly = wk.tile([128, NT], F32, tag="ly")
            lx = wk.tile([128, NT], F32, tag="lx")
            my = wk.tile([128, NT], F32, tag="my")
            mx = wk.tile([128, NT], F32, tag="mx")
            idxf = wk.tile([128, NT], F32, tag="idxf")
            wccL = big.tile([128, KK * 128], BF16, tag="wccL")
            wccR = big.tile([128, KK * 128], BF16, tag="wccR")

            nc.vector.tensor_tensor(py[:, :], dyv, bY[:, :], AO.add)
            nc.vector.tensor_scalar(py[:, :], py[:, :], clot[:, 0:1], None, AO.max)
            nc.vector.tensor_scalar(py[:, :], py[:, :], chit[:, 0:1], None, AO.min)
            nc.vector.tensor_scalar(tmp[:, :], py[:, :], 0.5, None, AO.subtract)
            nc.vector.tensor_copy(y0i[:, :], tmp[:, :])
            nc.vector.tensor_copy(y0f[:, :], y0i[:, :])
            nc.vector.tensor_tensor(px[:, :], dxv, bX[:, :], AO.add)
            nc.vector.tensor_scalar(px[:, :], px[:, :], 0.0, 131.0, AO.max, AO.min)
            nc.vector.tensor_scalar(tmp[:, :], px[:, :], 0.5, None, AO.subtract)
            nc.vector.tensor_copy(x0i[:, :], tmp[:, :])
            nc.vector.tensor_copy(x0f[:, :], x0i[:, :])
            nc.vector.tensor_tensor(ly[:, :], py[:, :], y0f[:, :], AO.subtract)
            nc.vector.tensor_tensor(lx[:, :], px[:, :], x0f[:, :], AO.subtract)
            nc.vector.tensor_scalar(my[:, :], ly[:, :], -1.0, 1.0, AO.mult, AO.add)
            nc.vector.tensor_scalar(mx[:, :], lx[:, :], -1.0, 1.0, AO.mult, AO.add)
            # corner weights: L planes (AL, BL) and R planes (AR, BR),
            # free layout (t,h,j): elem t*128 + h*2 + j
            for dst, j, a, bb_ in (
                (wccL, 0, my, mx), (wccL, 1, ly, mx),
                (wccR, 0, my, lx), (wccR, 1, ly, lx),
            ):
                dv = AP(dst[:].tensor, dst[:].offset + j,
                        [[KK * 128, 128], [2, 64], [128, 9]])
                nc.vector.tensor_tensor(dv, a[:, :], bb_[:, :], AO.mult)
            # gather index (fp32): idxf = y0*133 + x0, layout (t,h): elem t*64+h
            nc.vector.tensor_scalar(tmp[:, :], y0f[:, :], 133.0, None, AO.mult)
            iv = AP(idxf[:].tensor, idxf[:].offset, [[NT, 128], [1, 64], [64, 9]])
            nc.vector.tensor_tensor(iv, tmp[:, :], x0f[:, :], AO.add)

            # ---- 4. bounce bilinear weights to pixel-major via DRAM
            # wdram layout [t, plane(L/R), w, h, j]
            wdram = dr.tile([KK, WTAP], BF16, tag="wdram")
            for pl, wsrc in ((0, wccL), (1, wccR)):
                dv = AP(wdram[:].tensor, wdram[:].offset + pl * (NPIX * 2),
                        [[128, 128], [WTAP, 9], [1, 128]])
                sv = AP(wsrc[:].tensor, wsrc[:].offset,
                        [[KK * 128, 128], [128, 9], [1, 128]])
                nc.sync.dma_start(dv, sv)

            # ---- 5. wrapped gather indices iw [128, 9*512] u16 on PE
            # partitions 0:63: 2*(y0*133+x0) (left pair); 64:127: +2 (right)
            iw = big.tile([128, KK * 512], U16, tag="iw")
            for t in range(KK):
                tp2 = ps.tile([64, 128], F32, tag="ptT")
                nc.tensor.transpose(tp2[:, :], idxf[:, t * 64:(t + 1) * 64], idnF[:, :])
                Tt = wk.tile([64, 128], F32, tag="Tt")
                nc.scalar.copy(Tt[:, :], tp2[:, :])
                for hh in range(4):
                    pf = ps.tile([128, 128], F32, tag="ptF")
                    nc.tensor.matmul(pf[:, :], FOLDt[:, hh * 128:(hh + 1) * 128],
                                     Tt[:, :], start=True, stop=True)
                    ivA = AP(iw[:].tensor, iw[:].offset + t * 512 + hh,
                             [[KK * 512, 64], [4, 128]])
                    nc.vector.tensor_scalar(ivA, pf[0:64, :], 2.0, None, AO.mult)
                    ivB = AP(iw[:].tensor,
                             iw[:].offset + 64 * (KK * 512) + t * 512 + hh,
                             [[KK * 512, 64], [4, 128]])
                    nc.vector.tensor_scalar(ivB, pf[64:128, :], 2.0, 2.0,
                                            AO.mult, AO.add)

            wkpool.__exit__(None, None, None)
            head.__exit__(None, None, None)

            # ---- 6. main loop: quarters x taps
            mainps = tc.tile_pool(name="psm", bufs=2, space="PSUM")
            psw = mainps.__enter__()
            accpool = tc.tile_pool(name="psa", bufs=1, space="PSUM")
            psa = accpool.__enter__()
            outs = big.tile([Cout, NPIX], F32, tag="outs")
            datav = AP(Pt[:].tensor, Pt[:].offset,
                       [[2 * SLAB, 128], [2, SLAB], [1, 2]])
            for q in range(NQ):
                accs = [psa.tile([Cout, CHUNK], F32, name=f"acc{c}", tag=f"acc{c}")
                        for c in range(NCH)]
                for t in range(KK):
                    g2 = gp.tile([128, 2 * QS], BF16, tag="g2")
                    nc.gpsimd.indirect_copy(
                        g2[:].rearrange("p (n i) -> p n i", i=2), datav,
                        iw[:, t * 512 + q * 128: t * 512 + (q + 1) * 128], True)
                    wcm = wl.tile([2, 2 * QS], BF16, tag="wcm")
                    sv = AP(wdram[:].tensor,
                            wdram[:].offset + t * WTAP + q * (2 * QS),
                            [[NPIX * 2, 2], [1, 2 * QS]])
                    nc.sync.dma_start(wcm[:, :], sv)
                    for s_ in range(2 * QS // 512):
                        wra = psw.tile([128, 512], F32, tag="wra")
                        nc.tensor.matmul(wra[:, :], OH2t[:, :],
                                         wcm[:, s_ * 512:(s_ + 1) * 512],
                                         start=True, stop=True)
                        sl = slice(s_ * 512, (s_ + 1) * 512)
                        nc.vector.tensor_tensor(g2[:, sl], g2[:, sl], wra[:, :],
                                                AO.mult)
                    for c in range(NCH):
                        for j in range(2):
                            rhs = AP(g2[:].tensor,
                                     g2[:].offset + c * 2 * CHUNK + j,
                                     [[2 * QS, 128], [2, CHUNK]])
                            nc.tensor.matmul(
                                accs[c][:, :],
                                wmainT[:, t * Cout:(t + 1) * Cout], rhs,
                                start=(t == 0 and j == 0),
                                stop=(t == KK - 1 and j == 1))
                for c in range(NCH):
                    w0 = q * 32 + c * 8
                    dvo = AP(outs[:].tensor, outs[:].offset + w0,
                             [[NPIX, Cout], [1, 8], [128, 64]])
                    nc.scalar.activation(dvo, accs[c][:, :],
                                         mybir.ActivationFunctionType.Identity,
                                         bias=mbt[:, 0:1])
            accpool.__exit__(None, None, None)
            mainps.__exit__(None, None, None)

            # ---- 7. store output
            nc.sync.dma_start(out[:, :], outs[:, :])
    return nc


def make_host_consts():
    import ml_dtypes
    c = {}
    hs = np.arange(64)
    ts = np.arange(KK)
    ki = ts // 3
    kj = ts % 3
    c["bYp"] = np.tile((hs[:, None] + ki[None, :] + 7.0).reshape(1, NT),
                       (128, 1)).astype(np.float32)
    bx = (np.arange(128)[:, None, None] + kj[None, None, :] + 1.0)
    c["bXp"] = np.broadcast_to(bx, (128, 64, KK)).reshape(128, NT).astype(np.float32)
    c["idnf"] = np.eye(128, dtype=np.float32)
    c["idnb"] = np.eye(128).astype(ml_dtypes.bfloat16)
    fold = np.zeros((64, 4, 128), np.float32)
    for h in range(64):
        hh, k = h // 16, h % 16
        for g_ in range(8):
            fold[h, hh, 16 * g_ + k] = 1.0
    c["FOLD"] = fold.reshape(64, 4 * 128)
    oh2 = np.zeros((2, 128), np.float32)
    oh2[0, 0:64] = 1.0
    oh2[1, 64:128] = 1.0
    c["OH2"] = oh2.astype(ml_dtypes.bfloat16)
    return c


def make_in_maps(x, offset_w, offset_b, weight, bias):
    import ml_dtypes
    consts = make_host_consts()
    consts["woff"] = np.ascontiguousarray(
        offset_w.reshape(18, Cin, KK).transpose(1, 2, 0)
    ).reshape(Cin, KK * 18).astype(ml_dtypes.bfloat16)
    wm = np.ascontiguousarray(
        weight.reshape(Cout, Cin, KK).transpose(1, 2, 0)
    ).reshape(Cin, KK * Cout)
    consts["wmain"] = np.concatenate([wm, wm], axis=0).astype(ml_dtypes.bfloat16)
    consts["offb"] = offset_b.reshape(18, 1).astype(np.float32)
    consts["mainb"] = bias.reshape(Cout, 1).astype(np.float32)

    # padded image: rows/cols -2..130 (133x133), zeros outside
    xpad = np.zeros((B, Cin, PADW, PADW), np.float32)
    xpad[:, :, 2:2 + H, 2:2 + W] = x
    in_maps = []
    for core in range(8):
        b, half = core // 2, core % 2
        h0 = half * 64
        slab = np.zeros((Cin, SLABROWS, PADW), np.float32)
        glo = h0 - 8  # first slab row = padded-image row index glo+2
        lo = max(0, glo + 2)
        hi = min(PADW, glo + 2 + SLABROWS)
        slab[:, lo - (glo + 2):hi - (glo + 2), :] = xpad[b, :, lo:hi, :]
        flat = slab.reshape(Cin, SLAB)
        pair = np.zeros((Cin, SLAB, 2), np.float32)
        pair[:, :, 0] = flat
        pair[:, :-PADW, 1] = flat[:, PADW:]   # partner = same col, next row
        m = dict(consts)
        m["P"] = pair.reshape(Cin, 2 * SLAB).astype(ml_dtypes.bfloat16)
        lo_c = max(-2.0, h0 - 8.0) - h0 + 8.0
        hi_c = min(129.0, h0 + 74.0) - h0 + 8.0
        m["clo"] = np.full((128, 1), lo_c, np.float32)
        m["chi"] = np.full((128, 1), hi_c, np.float32)
        in_maps.append(m)
    return in_maps


_CACHED = {}

def kernel(x, offset_w, offset_b, weight, bias):
    from concourse.bass_utils import run_bass_kernel_spmd
    x = np.asarray(x, dtype=np.float32)
    offset_w = np.asarray(offset_w, dtype=np.float32)
    offset_b = np.asarray(offset_b, dtype=np.float32)
    weight = np.asarray(weight, dtype=np.float32)
    bias = np.asarray(bias, dtype=np.float32)
    if "nc" not in _CACHED:
        nc = build_program()
        split_waits(nc)
        _CACHED["nc"] = nc
    nc = _CACHED["nc"]
    in_maps = make_in_maps(x, offset_w, offset_b, weight, bias)
    res = run_bass_kernel_spmd(nc, in_maps, list(range(8)))
    out = np.zeros((B, Cout, H, W), dtype=np.float32)
    for core in range(8):
        b, half = core // 2, core % 2
        out[b, :, half * 64:(half + 1) * 64, :] = (
            res.results[core]["out"].reshape(Cout, 64, W))
    return out


# revision 3
# speedup vs baseline: 1.9097x; 1.0005x over previous
"""Deformable conv block (3x3) on 8 TRN2 NeuronCores — v2.1c.

Per core: one (batch, H-half): 64 output rows x 128 cols.
Key changes vs baseline:
  - all matmuls bf16 (4x PE speedup vs fp32)
  - row-pair-expanded slab: P2[c, 2j] = s[j], P2[c, 2j+1] = s[j+133], so one
    4-byte DGE descriptor fetches a (top, bottom) corner pair
  - one indirect_copy per (tap, quarter): partitions 0:63 fetch the left
    corner pair at idx, 64:127 the right pair at idx+2 -> K=128 main matmuls
  - wrapped gather indices built on PE (transpose + one-hot fold matmuls),
    no tiny strided SBUF->SBUF DMAs
  - all-taps vectorized coordinate math on DVE ([128,576] ops)
  - bilinear weights routed to pixel-major via one DRAM bounce
"""
import numpy as np

import concourse.bass as bass
import concourse.mybir as mybir
import concourse.tile as tile_mod
from concourse import tile
from concourse.vector_clock import ScopedClock

# ---------------------------------------------------------------------------
# Patch 1: this container's walrus accepts at most ONE sync wait per
# instruction; split the tile-exit drain's waits across preceding SP nops.
def _drain_and_barrier(self, tick_clock, wait_clock):
    nc = self.nc
    carriers = [nc.sync.nop(nofuse=True, hint=f"drainwait{i}") for i in range(32)]
    drain_inst = nc.sync.drain()
    wait_clock.add_sem_waits(drain_inst.ins, ScopedClock({None: tick_clock.global_clock}))
    si = drain_inst.ins.sync_info
    waits = list(si.on_wait or [])
    if len(waits) > 1:
        si.on_wait = waits[:1]
        for i, w in enumerate(waits[1:]):
            ci = carriers[i].ins
            if ci.sync_info is None:
                ci.sync_info = mybir.SyncInfo(on_wait=[w], on_update=[])
            else:
                ci.sync_info.on_wait = (ci.sync_info.on_wait or []) + [w]
    nc.all_engine_barrier()
    assert self.sems is not None
    popped = nc._tile_sem_poison_stack.pop()
    assert popped is self._sem_poison
    nc.clear_and_free_semaphores(list(self.sems.allocated().values()))
    nc.all_engine_barrier()

tile_mod.TileContext._drain_and_barrier = _drain_and_barrier

# Patch 2: split multi-wait instructions everywhere (same walrus limit).
_ctr = [0]

def _mk_nop(engine, wait):
    _ctr[0] += 1
    nop = mybir.InstNoOp(name=f"WSPLIT-{_ctr[0]}", ins=[], outs=[])
    nop.engine = engine
    nop.sync_info = mybir.SyncInfo(on_wait=[wait], on_update=[])
    return nop

def split_waits(nc):
    n = 0
    for fn in nc.m.functions:
        for bb in fn.blocks:
            insts = list(bb.instructions)
            outl, changed = [], False
            for inst in insts:
                si = inst.sync_info
                if si is not None and si.on_wait and len(si.on_wait) > 1:
                    waits = list(si.on_wait)
                    for w in waits[:-1]:
                        nop = _mk_nop(inst.engine, w)
                        nc.register_instruction(nop, overwrite=True)
                        outl.append(nop)
                        n += 1
                    si.on_wait = waits[-1:]
                    inst.sync_info = si
                    changed = True
                outl.append(inst)
            if changed:
                bb.instructions = outl
    return n

# ---------------------------------------------------------------------------
F32 = mybir.dt.float32
BF16 = mybir.dt.bfloat16
I32 = mybir.dt.int32
U16 = mybir.dt.uint16
AO = mybir.AluOpType
AP = bass.AP

B, Cin, Cout, H, W = 4, 64, 64, 128, 128
KK = 9
PADW = 133
SLABROWS = 84
SLAB = SLABROWS * PADW          # 11172 slab elements per channel
NPIX = 64 * W                   # 8192 output pixels per core
NT = 576                        # 64 h x 9 taps coordinate lanes
NQ = 4                          # quarters of the pixel space (by w)
QS = NPIX // NQ                 # 2048 slots per quarter
CHUNK = 512                     # slots per PSUM accumulator chunk
NCH = QS // CHUNK               # 4 chunks per quarter
WTAP = 2 * NPIX * 2             # weight-dram elements per tap (2 planes x 2)


def build_program():
    nc = bass.Bass()
    Pp = nc.declare_dram_parameter("P", [Cin, 2 * SLAB], BF16, isOutput=False)
    woff = nc.declare_dram_parameter("woff", [Cin, KK * 18], BF16, isOutput=False)
    wmain = nc.declare_dram_parameter("wmain", [128, KK * Cout], BF16, isOutput=False)
    bYp = nc.declare_dram_parameter("bYp", [128, NT], F32, isOutput=False)
    bXp = nc.declare_dram_parameter("bXp", [128, NT], F32, isOutput=False)
    idnf = nc.declare_dram_parameter("idnf", [128, 128], F32, isOutput=False)
    idnb = nc.declare_dram_parameter("idnb", [128, 128], BF16, isOutput=False)
    FW = nc.declare_dram_parameter("FW", [128, 8 * 128], F32, isOutput=False)
    blkc = nc.declare_dram_parameter("blkc", [128, NT], F32, isOutput=False)
    OH2 = nc.declare_dram_parameter("OH2", [2, 128], BF16, isOutput=False)
    offb = nc.declare_dram_parameter("offb", [18, 1], F32, isOutput=False)
    mainb = nc.declare_dram_parameter("mainb", [Cout, 1], F32, isOutput=False)
    clo = nc.declare_dram_parameter("clo", [128, 1], F32, isOutput=False)
    chi = nc.declare_dram_parameter("chi", [128, 1], F32, isOutput=False)
    out = nc.declare_dram_parameter("out", [Cout, NPIX], F32, isOutput=True)

    with tile.TileContext(nc) as tc:
        with (
            tc.tile_pool(name="big", bufs=1) as big,
            tc.tile_pool(name="g", bufs=4) as gp,
            tc.tile_pool(name="wl", bufs=3) as wl,
            tc.tile_pool(name="dr", bufs=1, space="DRAM") as dr,
        ):
            Pt = big.tile([128, 2 * SLAB], BF16, tag="Pt")
            nc.sync.dma_start(Pt[0:64, :], Pp[:, :])
            nc.sync.dma_start(Pt[64:128, :], Pp[:, :])
            woffT = big.tile([Cin, KK * 18], BF16, tag="woff")
            nc.sync.dma_start(woffT[:, :], woff[:, :])
            wmainT = big.tile([128, KK * Cout], BF16, tag="wmain")
            nc.sync.dma_start(wmainT[:, :], wmain[:, :])
            bY = big.tile([128, NT], F32, tag="bY")
            nc.sync.dma_start(bY[:, :], bYp[:, :])
            bX = big.tile([128, NT], F32, tag="bX")
            nc.sync.dma_start(bX[:, :], bXp[:, :])
            idnF = big.tile([128, 128], F32, tag="idnF")
            nc.sync.dma_start(idnF[:, :], idnf[:, :])
            idnB = big.tile([128, 128], BF16, tag="idnB")
            nc.sync.dma_start(idnB[:, :], idnb[:, :])
            FWt = big.tile([128, 8 * 128], F32, tag="FW")
            nc.sync.dma_start(FWt[:, :], FW[:, :])
            blkct = big.tile([128, NT], F32, tag="blkc")
            nc.sync.dma_start(blkct[:, :], blkc[:, :])
            OH2t = big.tile([2, 128], BF16, tag="OH2")
            nc.sync.dma_start(OH2t[:, :], OH2[:, :])
            obt = big.tile([18, 1], F32, tag="ob")
            nc.sync.dma_start(obt[:, :], offb[:, :])
            mbt = big.tile([Cout, 1], F32, tag="mb")
            nc.sync.dma_start(mbt[:, :], mainb[:, :])
            clot = big.tile([128, 1], F32, tag="clo")
            nc.sync.dma_start(clot[:, :], clo[:, :])
            chit = big.tile([128, 1], F32, tag="chi")
            nc.sync.dma_start(chit[:, :], chi[:, :])

            head = tc.tile_pool(name="psh", bufs=2, space="PSUM")
            ps = head.__enter__()
            wkpool = tc.tile_pool(name="wk", bufs=1)
            wk = wkpool.__enter__()

            # ---- 1. offset conv -> offs bf16 [18, 8192] (pixel-major h*128+w)
            # x values sit at even offsets of the row-pair slab (stride 2)
            offs = wk.tile([18, NPIX], BF16, tag="offs")
            for ch in range(16):
                pt = ps.tile([18, 512], F32, tag="pofs")
                for t in range(KK):
                    ki, kj = t // 3, t % 3
                    off0 = 2 * ((ch * 4 + ki + 7) * PADW + (kj + 1))
                    rhs = AP(Pt[:].tensor, Pt[:].offset + off0,
                             [[2 * SLAB, 64], [2 * PADW, 4], [2, 128]])
                    nc.tensor.matmul(pt[:, :], woffT[:, t * 18:(t + 1) * 18], rhs,
                                     start=(t == 0), stop=(t == KK - 1))
                nc.scalar.activation(offs[:, ch * 512:(ch + 1) * 512], pt[:, :],
                                     mybir.ActivationFunctionType.Identity,
                                     bias=obt[:, 0:1])

            # ---- 2. transpose offsets -> offT f32 [128w, (64h x 18ch)]
            offT = big.tile([128, 64 * 18], F32, tag="offT")
            for h in range(64):
                tp = ps.tile([128, 18], BF16, tag="ptr")
                nc.tensor.transpose(tp[:, :], offs[:, h * 128:(h + 1) * 128],
                                    idnB[0:18, 0:18])
                ov = AP(offT[:].tensor, offT[:].offset + h * 18,
                        [[64 * 18, 128], [1, 18]])
                nc.scalar.copy(ov, tp[:, :])

            # ---- 3. all-taps coordinate pipeline on [128, 576] (f = h*9 + t)
            dyv = AP(offT[:].tensor, offT[:].offset + 0, [[64 * 18, 128], [18, 64], [2, 9]])
            dxv = AP(offT[:].tensor, offT[:].offset + 1, [[64 * 18, 128], [18, 64], [2, 9]])
            py = wk.tile([128, NT], F32, tag="py")
            px = wk.tile([128, NT], F32, tag="px")
            tmp = wk.tile([128, NT], F32, tag="tmp")
            y0i = wk.tile([128, NT], I32, tag="y0i")
            x0i = wk.tile([128, NT], I32, tag="x0i")
            y0f = wk.tile([128, NT], F32, tag="y0f")
            x0f = wk.tile([128, NT], F32, tag="x0f")
            ly = wk.tile([128, NT], F32, tag="ly")
            lx = wk.tile([128, NT], F32, tag="lx")
            my = wk.tile([128, NT], F32, tag="my")
            mx = wk.tile([128, NT], F32, tag="mx")
            idxf = wk.tile([128, NT], F32, tag="idxf")
            wccL = big.tile([128, KK * 128], BF16, tag="wccL")
            wccR = big.tile([128, KK * 128], BF16, tag="wccR")

            nc.vector.tensor_tensor(py[:, :], dyv, bY[:, :], AO.add)
            nc.vector.tensor_scalar(py[:, :], py[:, :], clot[:, 0:1], None, AO.max)
            nc.vector.tensor_scalar(py[:, :], py[:, :], chit[:, 0:1], None, AO.min)
            nc.vector.tensor_scalar(tmp[:, :], py[:, :], 0.5, None, AO.subtract)
            nc.vector.tensor_copy(y0i[:, :], tmp[:, :])
            nc.vector.tensor_copy(y0f[:, :], y0i[:, :])
            nc.vector.tensor_tensor(px[:, :], dxv, bX[:, :], AO.add)
            nc.vector.tensor_scalar(px[:, :], px[:, :], 0.0, 131.0, AO.max, AO.min)
            nc.vector.tensor_scalar(tmp[:, :], px[:, :], 0.5, None, AO.subtract)
            nc.vector.tensor_copy(x0i[:, :], tmp[:, :])
            nc.vector.tensor_copy(x0f[:, :], x0i[:, :])
            nc.vector.tensor_tensor(ly[:, :], py[:, :], y0f[:, :], AO.subtract)
            nc.vector.tensor_tensor(lx[:, :], px[:, :], x0f[:, :], AO.subtract)
            nc.vector.tensor_scalar(my[:, :], ly[:, :], -1.0, 1.0, AO.mult, AO.add)
            nc.vector.tensor_scalar(mx[:, :], lx[:, :], -1.0, 1.0, AO.mult, AO.add)
            # corner weights: L planes (AL, BL) and R planes (AR, BR),
            # free layout (t,h,j): elem t*128 + h*2 + j
            for dst, j, a, bb_ in (
                (wccL, 0, my, mx), (wccL, 1, ly, mx),
                (wccR, 0, my, lx), (wccR, 1, ly, lx),
            ):
                dv = AP(dst[:].tensor, dst[:].offset + j,
                        [[KK * 128, 128], [2, 64], [128, 9]])
                nc.vector.tensor_tensor(dv, a[:, :], bb_[:, :], AO.mult)
            # gather index (fp32): idxf = y0*133 + x0, layout (t,h): elem t*64+h
            nc.vector.tensor_scalar(tmp[:, :], y0f[:, :], 133.0, None, AO.mult)
            nc.vector.tensor_tensor(tmp[:, :], tmp[:, :], blkct[:, :], AO.add)
            iv = AP(idxf[:].tensor, idxf[:].offset, [[NT, 128], [1, 64], [64, 9]])
            nc.vector.tensor_tensor(iv, tmp[:, :], x0f[:, :], AO.add)
            nc.vector.tensor_scalar(idxf[:, :], idxf[:, :], 0.0, 3589.0,
                                    AO.max, AO.min)

            # ---- 4. bounce bilinear weights to pixel-major via DRAM
            # wdram layout [t, plane(L/R), w, h, j]
            wdram = dr.tile([KK, WTAP], BF16, tag="wdram")
            for pl, wsrc in ((0, wccL), (1, wccR)):
                dv = AP(wdram[:].tensor, wdram[:].offset + pl * (NPIX * 2),
                        [[128, 128], [WTAP, 9], [1, 128]])
                sv = AP(wsrc[:].tensor, wsrc[:].offset,
                        [[KK * 128, 128], [128, 9], [1, 128]])
                nc.sync.dma_start(dv, sv)

            # ---- 5. wrapped gather indices iw [128, 9*512] u16 on PE
            # slot n (block blk) = h_local*128 + w; partition k = w%16,
            # word = h*8 + w//16 (word slice [blk*64, blk*64+64) per block)
            # lower 64 partitions: pair-slot idx (left col); upper: +1
            iw = big.tile([128, KK * 512], U16, tag="iw")
            for t in range(KK):
                for wq in range(8):
                    pf = ps.tile([128, 64], F32, tag="ptF")
                    nc.tensor.matmul(pf[:, :], FWt[:, wq * 128:(wq + 1) * 128],
                                     idxf[:, t * 64:(t + 1) * 64],
                                     start=True, stop=True)
                    ivA = AP(iw[:].tensor, iw[:].offset + t * 512 + wq,
                             [[KK * 512, 64], [8, 64]])
                    nc.vector.tensor_copy(ivA, pf[0:64, :])
                    ivB = AP(iw[:].tensor,
                             iw[:].offset + 64 * (KK * 512) + t * 512 + wq,
                             [[KK * 512, 64], [8, 64]])
                    nc.vector.tensor_scalar(ivB, pf[64:128, :], 1.0, None, AO.add)

            wkpool.__exit__(None, None, None)
            head.__exit__(None, None, None)

            # ---- 6. main loop: quarters x taps
            mainps = tc.tile_pool(name="psm", bufs=4, space="PSUM")
            psw = mainps.__enter__()
            accpool = tc.tile_pool(name="psa", bufs=1, space="PSUM")
            psa = accpool.__enter__()
            outs = big.tile([Cout, NPIX], F32, tag="outs")
            datav = AP(Pt[:].tensor, Pt[:].offset,
                       [[2 * SLAB, 128], [2, SLAB], [1, 2]])
            for q in range(NQ):
                accs = [psa.tile([Cout, CHUNK], F32, name=f"acc{c}", tag=f"acc{c}")
                        for c in range(NCH)]
                for t in range(KK):
                    g2 = gp.tile([128, 2 * QS], BF16, tag="g2")
                    nc.gpsimd.indirect_copy(
                        g2[:].rearrange("p (n i) -> p n i", i=2), datav,
                        iw[:, t * 512 + q * 128: t * 512 + (q + 1) * 128], True)
                    wcm = wl.tile([2, 2 * QS], BF16, tag="wcm")
                    sv = AP(wdram[:].tensor,
                            wdram[:].offset + t * WTAP + q * (2 * QS),
                            [[NPIX * 2, 2], [1, 2 * QS]])
                    nc.sync.dma_start(wcm[:, :], sv)
                    for s_ in range(2 * QS // 512):
                        wra = psw.tile([128, 512], F32, tag="wra")
                        nc.tensor.matmul(wra[:, :], OH2t[:, :],
                                         wcm[:, s_ * 512:(s_ + 1) * 512],
                                         start=True, stop=True)
                        sl = slice(s_ * 512, (s_ + 1) * 512)
                        nc.vector.tensor_tensor(g2[:, sl], g2[:, sl], wra[:, :],
                                                AO.mult)
                    for c in range(NCH):
                        for j in range(2):
                            rhs = AP(g2[:].tensor,
                                     g2[:].offset + c * 2 * CHUNK + j,
                                     [[2 * QS, 128], [2, CHUNK]])
                            nc.tensor.matmul(
                                accs[c][:, :],
                                wmainT[:, t * Cout:(t + 1) * Cout], rhs,
                                start=(t == 0 and j == 0),
                                stop=(t == KK - 1 and j == 1))
                for c in range(NCH):
                    w0 = q * 32 + c * 8
                    dvo = AP(outs[:].tensor, outs[:].offset + w0,
                             [[NPIX, Cout], [1, 8], [128, 64]])
                    nc.scalar.activation(dvo, accs[c][:, :],
                                         mybir.ActivationFunctionType.Identity,
                                         bias=mbt[:, 0:1])
            accpool.__exit__(None, None, None)
            mainps.__exit__(None, None, None)

            # ---- 7. store output
            nc.sync.dma_start(out[:, :], outs[:, :])
    return nc


def make_host_consts():
    import ml_dtypes
    c = {}
    hs = np.arange(64)
    ts = np.arange(KK)
    ki = ts // 3
    kj = ts % 3
    c["bYp"] = np.tile((hs[:, None] + ki[None, :] + 7.0).reshape(1, NT),
                       (128, 1)).astype(np.float32)
    bx = (np.arange(128)[:, None, None] + kj[None, None, :] + 1.0)
    c["bXp"] = np.broadcast_to(bx, (128, 64, KK)).reshape(128, NT).astype(np.float32)
    c["idnf"] = np.eye(128, dtype=np.float32)
    c["idnb"] = np.eye(128).astype(ml_dtypes.bfloat16)
    fw = np.zeros((128, 8, 128), np.float32)
    for w in range(128):
        wq, k = w // 16, w % 16
        for g_ in range(8):
            fw[w, wq, 16 * g_ + k] = 1.0
    c["FW"] = fw.reshape(128, 8 * 128)
    blkc = np.zeros((64, KK), np.float32)
    for h in range(64):
        blkc[h, :] = -1064.0 * (h // 8)
    c["blkc"] = np.tile(blkc.reshape(1, NT), (128, 1))
    oh2 = np.zeros((2, 128), np.float32)
    oh2[0, 0:64] = 1.0
    oh2[1, 64:128] = 1.0
    c["OH2"] = oh2.astype(ml_dtypes.bfloat16)
    return c


def make_in_maps(x, offset_w, offset_b, weight, bias):
    import ml_dtypes
    consts = make_host_consts()
    consts["woff"] = np.ascontiguousarray(
        offset_w.reshape(18, Cin, KK).transpose(1, 2, 0)
    ).reshape(Cin, KK * 18).astype(ml_dtypes.bfloat16)
    wm = np.ascontiguousarray(
        weight.reshape(Cout, Cin, KK).transpose(1, 2, 0)
    ).reshape(Cin, KK * Cout)
    consts["wmain"] = np.concatenate([wm, wm], axis=0).astype(ml_dtypes.bfloat16)
    consts["offb"] = offset_b.reshape(18, 1).astype(np.float32)
    consts["mainb"] = bias.reshape(Cout, 1).astype(np.float32)

    # padded image: rows/cols -2..130 (133x133), zeros outside
    xpad = np.zeros((B, Cin, PADW, PADW), np.float32)
    xpad[:, :, 2:2 + H, 2:2 + W] = x
    in_maps = []
    for core in range(8):
        b, half = core // 2, core % 2
        h0 = half * 64
        slab = np.zeros((Cin, SLABROWS, PADW), np.float32)
        glo = h0 - 8  # first slab row = padded-image row index glo+2
        lo = max(0, glo + 2)
        hi = min(PADW, glo + 2 + SLABROWS)
        slab[:, lo - (glo + 2):hi - (glo + 2), :] = xpad[b, :, lo:hi, :]
        flat = slab.reshape(Cin, SLAB)
        pair = np.zeros((Cin, SLAB, 2), np.float32)
        pair[:, :, 0] = flat
        pair[:, :-PADW, 1] = flat[:, PADW:]   # partner = same col, next row
        m = dict(consts)
        m["P"] = pair.reshape(Cin, 2 * SLAB).astype(ml_dtypes.bfloat16)
        lo_c = max(-2.0, h0 - 8.0) - h0 + 8.0
        hi_c = min(129.0, h0 + 74.0) - h0 + 8.0
        m["clo"] = np.full((128, 1), lo_c, np.float32)
        m["chi"] = np.full((128, 1), hi_c, np.float32)
        in_maps.append(m)
    return in_maps


_CACHED = {}

def kernel(x, offset_w, offset_b, weight, bias):
    from concourse.bass_utils import run_bass_kernel_spmd
    x = np.asarray(x, dtype=np.float32)
    offset_w = np.asarray(offset_w, dtype=np.float32)
    offset_b = np.asarray(offset_b, dtype=np.float32)
    weight = np.asarray(weight, dtype=np.float32)
    bias = np.asarray(bias, dtype=np.float32)
    if "nc" not in _CACHED:
        nc = build_program()
        split_waits(nc)
        _CACHED["nc"] = nc
    nc = _CACHED["nc"]
    in_maps = make_in_maps(x, offset_w, offset_b, weight, bias)
    res = run_bass_kernel_spmd(nc, in_maps, list(range(8)))
    out = np.zeros((B, Cout, H, W), dtype=np.float32)
    for core in range(8):
        b, half = core // 2, core % 2
        out[b, :, half * 64:(half + 1) * 64, :] = (
            res.results[core]["out"].reshape(Cout, 64, W))
    return out
